# revision 30
# baseline (speedup 1.0000x reference)
"""Grouped GEMM (MoE routing) Trainium2 kernel.

Expert-parallel across 8 NeuronCores, size-sorted slot assignment
(slot s holds the experts of size-rank [8s, 8s+8), one per core; the
SPMD per-slot moving width cap_s is the rank-group max, so padding is
the rank gap only — no 128-row M-tile rounding).

Weight-stationary layout computing out^T: per (slot, dout-chunk n) the
PE accumulates psum[128, cap] = sum_k w[k-slab, n-chunk].T @ xt[k-slab]
over the 20 contraction slabs; tokens are the moving operand so the
stream cost is the exact token count.  Both operands are fp8 e3m4
(4 mantissa bits: half the quantization noise of e4m3; weights x64
scale), PSUM fp32, outputs fp16 (unscaled on host) — ~41 MB of HBM
traffic/core vs ~123 us of PE stream, right at the ridge.

Schedule: slots 0,1 run alone; the LDWEIGHTS-bound small slots (cap
< ~95, where the ~95 ns weight load outpaces the token stream) are
group-interleaved with a big carrier slot so their weight loads hide
under the carrier's long matmul streams.  Weights ride the scalar
HWDGE ring (whole-expert 4.26 MB DMAs; slot 0 in 4 pieces to fill the
pipeline), xt the sync ring (all 8 slots up front), outputs SWDGE so
no input-prefetch doorbell ever queues behind a compute-gated
instruction.  PSUM rotates through all 8 banks; drains are
Vector-only for the same reason.
"""
import ml_dtypes
import numpy as np

import concourse.bass as bass
import concourse.mybir as mybir
import concourse.tile as tile
from concourse import bacc
from concourse.bass_utils import run_bass_kernel_spmd

G, T, DIN, DOUT = 64, 8192, 2560, 1664
NCORES = 8
EPC = G // NCORES       # expert slots per core
KC = DIN // 128         # 20 contraction slabs
NCH = DOUT // 128       # 13 dout chunks
WS = 64.0               # weight quant scale (power of two; sigma~1.3 in e3m4)
F8MAX = 15.0            # clip inside e3m4 normal range

_cache = {}


def _build(caps):
    offs = np.concatenate([[0], np.cumsum(caps)]).astype(int)
    sumcap = int(offs[-1])
    nc = bacc.Bacc(trn_type="TRN2", debug=False)
    bf16 = mybir.dt.bfloat16
    fp8 = mybir.dt.float8e3
    fp16 = mybir.dt.float16
    f32 = mybir.dt.float32
    xt = nc.dram_tensor("xt", [128, KC * sumcap], fp8, kind="ExternalInput").ap()
    w = nc.dram_tensor("w", [EPC, 128, NCH * KC * 128], fp8, kind="ExternalInput").ap()
    out = nc.dram_tensor("out", [128, NCH * sumcap], fp16, kind="ExternalOutput").ap()
    with tile.TileContext(nc) as tc:
        with (
            tc.tile_pool(name="xp", bufs=8) as xp,
            tc.tile_pool(name="wp", bufs=4) as wp,
            tc.tile_pool(name="op", bufs=4) as op,
            tc.tile_pool(name="pp", bufs=8, space="PSUM") as pp,
        ):
            # Processing schedule: LDW-bound small slots (cap < ~95:
            # LDWEIGHTS outpaces the token stream) are group-interleaved
            # with a big partner so their weight loads hide under the
            # partner's long matmul streams; pair compute shrinks down the
            # schedule so the uniform 8.5MB/pair weight stream stays ahead.
            sched = [(0,), (1,), (2, 5), (3, 6), (4, 7)]
            xt_sbs, w_sbs, o_sbs = {}, {}, {}

            def load_slot(s):
                cap, off = int(caps[s]), int(offs[s])
                xt_sb = xp.tile([128, KC * cap], fp8, tag="xt", name=f"xt{s}")
                xcuts = [0, cap, 5 * cap, KC * cap] if s == 0 else [0, KC * cap]
                for lo, hi in zip(xcuts, xcuts[1:]):
                    nc.sync.dma_start(
                        xt_sb[:, lo:hi], xt[:, KC * off + lo:KC * off + hi]
                    )
                w_sb = wp.tile([128, NCH * KC * 128], fp8, tag="w", name=f"w{s}")
                if s == 0:
                    wcuts = [0, 1280, 2 * 2560, 5 * 2560, 9 * 2560, NCH * 2560]
                else:
                    wcuts = [0, NCH * 2560]
                for lo, hi in zip(wcuts, wcuts[1:]):
                    nc.scalar.dma_start(w_sb[:, lo:hi], w[s, :, lo:hi])
                xt_sbs[s], w_sbs[s] = xt_sb, w_sb

            def group(s, n):
                cap = int(caps[s])
                ps_t = pp.tile([128, cap], f32, tag="ps", name=f"ps_{s}_{n}")
                for k in range(KC):
                    nc.tensor.matmul(
                        ps_t[:],
                        w_sbs[s][:, n * 2560 + k * 128:n * 2560 + (k + 1) * 128],
                        xt_sbs[s][:, k * cap:(k + 1) * cap],
                        start=(k == 0),
                        stop=(k == KC - 1),
                    )
                nc.vector.tensor_copy(
                    o_sbs[s][:, n * cap:(n + 1) * cap], ps_t[:]
                )

            for item in sched:
                for s in item:
                    load_slot(s)
                    o_sbs[s] = op.tile(
                        [128, NCH * int(caps[s])], fp16, tag="o", name=f"o{s}"
                    )
                for n in range(NCH):
                    for s in item:
                        group(s, n)
                for s in item:
                    cap, off = int(caps[s]), int(offs[s])
                    half = (NCH * cap) // 2
                    nc.gpsimd.dma_start(
                        out[:, NCH * off:NCH * off + half], o_sbs[s][:, :half]
                    )
                    nc.gpsimd.dma_start(
                        out[:, NCH * off + half:NCH * off + NCH * cap],
                        o_sbs[s][:, half:NCH * cap],
                    )
    nc.compile()
    return nc


def _run(inputs, trace=False):
    x = np.asarray(inputs["input"], dtype=np.float32)
    w = np.ascontiguousarray(np.asarray(inputs["weight"], dtype=np.float32))
    counts = np.asarray(inputs["tokens_per_expert"], dtype=np.int64)
    starts = np.concatenate([[0], np.cumsum(counts)[:-1]])

    order = np.argsort(-counts, kind="stable")  # experts by size rank
    # slot s, core c -> expert order[s*NCORES + c]; cap = rank-group max
    caps = tuple(
        int(np.ceil(max(2, counts[order[s * NCORES:(s + 1) * NCORES]].max()) / 2) * 2)
        for s in range(EPC)
    )
    offs = np.concatenate([[0], np.cumsum(caps)]).astype(int)
    sumcap = int(offs[-1])

    if caps not in _cache:
        _cache[caps] = _build(caps)
    nc = _cache[caps]

    wq = np.clip(w * WS, -F8MAX, F8MAX).astype(ml_dtypes.float8_e3m4)
    xb = x.astype(ml_dtypes.float8_e3m4)

    in_maps = []
    for c in range(NCORES):
        xt_pack = np.zeros((128, KC * sumcap), dtype=ml_dtypes.float8_e3m4)
        w_pack = np.empty((EPC, 128, NCH * KC * 128), dtype=ml_dtypes.float8_e3m4)
        for s in range(EPC):
            g = int(order[s * NCORES + c])
            cnt = int(counts[g])
            cap = int(caps[s])
            if cnt:
                # xt slab [p, k, t] = x[start+t, k*128+p]
                xs = xb[starts[g]:starts[g] + cnt].T.reshape(KC, 128, cnt)
                xt_pack[:, KC * offs[s]:KC * offs[s] + KC * cap].reshape(
                    128, KC, cap
                )[:, :, :cnt] = xs.transpose(1, 0, 2)
            # w line [p, n*2560 + k*128 + c2] = wq[g, k*128+p, n*128+c2]
            w_pack[s] = (
                wq[g].reshape(KC, 128, NCH, 128)
                .transpose(1, 2, 0, 3)
                .reshape(128, NCH * KC * 128)
            )
        in_maps.append({"xt": xt_pack, "w": w_pack})

    kw = {"trace_cores": list(range(NCORES))} if trace else {}
    res = run_bass_kernel_spmd(nc, in_maps, core_ids=list(range(NCORES)),
                               trace=trace, **kw)

    out = np.empty((T, DOUT), dtype=np.float32)
    inv = 1.0 / WS
    for c in range(NCORES):
        ot = res.results[c]["out"]  # [128, NCH*sumcap] fp16, slot-major blocks
        for s in range(EPC):
            g = int(order[s * NCORES + c])
            cnt = int(counts[g])
            cap = int(caps[s])
            if cnt:
                blk = ot[:, NCH * offs[s]:NCH * offs[s] + NCH * cap].reshape(
                    128, NCH, cap
                )[:, :, :cnt].astype(np.float32)
                # out[t, n*128+p] = blk[p, n, t]
                out[starts[g]:starts[g] + cnt] = (
                    blk.transpose(2, 1, 0).reshape(cnt, DOUT) * inv
                )
    return out, res


def kernel(**inputs) -> np.ndarray:
    return _run(inputs)[0]
